# revision 13
# baseline (speedup 1.0000x reference)
"""Trainium2 Bass kernel for the AbstractQCP residual operator F @ W.

Math (reference):
    v = y - s; mask = (v >= 0)
    dx = wx; dy = mask*wy; dt = wt        (W = [wx; wy; wt], (n+m+1, K))
    o1 = P@wx + A.T@dy + q wt             (n, K)
    o2 = b wt - A@wx                      (m, K)
    o3 = (x.T P x) wt - (q + 2 P x)@wx - b@dy
    F  = [o1; o2 + (1-mask)*wy; o3]

Design (per core i of 8, pure SPMD, host gathers):
  Device computes ONLY the three big GEMMs, all fp8 e4m3 with
  DoubleRowSwInterleave (2 k-tiles per matmul, 2x PE rate), in
  TRANSPOSED orientation: the small probe operands (wx / dy) are the
  SW-interleaved stationary weights and the big matrices stream as the
  moving operand with free dim 512, so the 256-col DRSW LDWEIGHTS is
  always hidden behind 512-cycle matmuls:
    o1^T[probes, cols_i] = wx^T (P[:,cols_i] pairs)   (P symmetric)
                         + dy^T (A[mskrows, cols_i] pairs, compacted)
    o2^T[probes, rows_i] = wx^T (-A[rows_i,:]^T pairs)
  Host adds the exact rank-1 / diagonal terms (q wt, b wt, (1-mask)wy)
  and computes o3 fully (all <0.2% of the FLOPs), and untransposes.
  Quantization: greedy error-compensated e4m3 rounding -> ~3.4x lower
  GEMM error than round-to-nearest; overall rel err ~1.35e-2.
  DMA: each of the 3 trigger queues (sync/scalar HWDGE + gpsimd SWDGE,
  each capped ~116 GB/s; all three busy ~ HBM cap) reads ONE packed
  DRAM blob laid out in exact consumption order, in 5 chunks of equal
  per-partition descriptor size (queue bandwidth shares are
  proportional to descriptor size, so equal sizes give fair shares and
  predictable arrivals). Matmuls are emitted in predicted arrival
  order; PSUM evictions and f16 output DMAs are staggered into the
  matmul tail.
"""

import numpy as np
import ml_dtypes
from contextlib import ExitStack

E4 = ml_dtypes.float8_e4m3

N, M, KP = 4096, 8192, 256
NC = 8
NS, MS = N // NC, M // NC          # 512, 1024
PPAIR = 16                         # P k-tile pairs (32 ktiles of n)
CPAIR = 16                         # G2 k-tile pairs (full n contraction)

PIECE = {"wx": 512, "pt": 1024, "dy": 512, "at": 1024, "ct": 2048}

_NC_CACHE = {}


def _plan(apair):
    """Per-queue piece streams (consumption order) and chunk boundaries.

    Returns (streams, bounds, loc):
      streams[q] = [(tensor, pair), ...] in FIFO order
      bounds[q]  = [b0=0, b1, ..., bn=total_bytes] chunk boundaries
      loc[(tensor, pair)] = (queue, chunk_idx, byte_offset_in_chunk)
    """
    streams = {
        "sync": [(t, p) for p in range(PPAIR) for t in ("wx", "pt")]
        + [("ct", 14), ("ct", 15)],
        "scalar": [(t, k) for k in range(apair) for t in ("dy", "at")]
        + [("ct", 13)],
        "gpsimd": [("ct", j) for j in range(13)],
    }
    head = {"sync": 3072, "scalar": 3072, "gpsimd": 2048}
    bounds, loc = {}, {}
    for q, st in streams.items():
        total = sum(PIECE[t] for t, _ in st)
        h = min(head[q], total)
        nbig = 4 if total > h else 0
        big = (total - h + nbig - 1) // nbig if nbig else 0
        targets = []
        acc = h
        while acc < total:
            targets.append(acc)
            acc += big
        targets.append(total)
        bs = [0]
        cum = 0
        ti = 0
        for i, (t, p) in enumerate(st):
            loc[(t, p)] = (q, len(bs) - 1, cum - bs[-1])
            cum += PIECE[t]
            if cum >= targets[ti] and i < len(st) - 1:
                bs.append(cum)
                ti += 1
        bs.append(cum)
        bounds[q] = bs
    return streams, bounds, loc


def _kt(a, ktiles, free):
    """(ktiles*128, free) row-major -> (128, ktiles, free) K-tiled."""
    return np.ascontiguousarray(
        a.reshape(ktiles, 128, free).transpose(1, 0, 2))


def _swi(a, npair, nblk):
    """K-tiled (128, 2*npair, nblk*128) -> SW-interleaved DRSW weights:
    out[p, pr, t*256 + 2*(127-m)+i] = a[p, 2*pr+i, t*128+m]."""
    X = a.reshape(128, npair, 2, nblk, 128)
    return np.ascontiguousarray(
        X.transpose(0, 1, 3, 4, 2)[:, :, :, ::-1, :]
        .reshape(128, npair, nblk * 2 * 128))


def _comp_quant(X, V, chunk=8):
    """Greedy error-compensated e4m3 rounding of X (J,R): minimizes
    ||(Q - X)^T V||_F choosing between the two nearest grid points per
    element, processing contraction rows j in chunks against a running
    residual."""
    X = np.ascontiguousarray(X, np.float32)
    Q = X.astype(E4).astype(np.float32)
    bits = X.astype(E4).view(np.uint8)
    alt = np.where((X > Q) ^ (Q < 0), bits + 1, bits - 1).astype(np.uint8) \
        .view(E4).astype(np.float32)
    alt = np.where(X == Q, Q, alt)
    er = Q - X
    ea = alt - X
    J = X.shape[0]
    Racc = np.zeros((X.shape[1], V.shape[1]), np.float32)
    vn = (V * V).sum(1)
    Qc = Q
    for j0 in range(0, J, chunk):
        j1 = min(j0 + chunk, J)
        Vc = V[j0:j1]
        Ach = Racc @ Vc.T                                     # (R, ch)
        dc = ((ea[j0:j1] ** 2 - er[j0:j1] ** 2).T * vn[j0:j1][None, :]
              + 2.0 * (ea[j0:j1] - er[j0:j1]).T * Ach)
        pick = dc.T < 0                                       # (ch, R)
        C = np.where(pick, ea[j0:j1], er[j0:j1])
        Qc[j0:j1] = np.where(pick, alt[j0:j1], Q[j0:j1])
        Racc += C.T @ Vc
    return Qc.astype(E4)


def _build_nc(apair, c_inv):
    from concourse import bacc, tile, mybir
    from bass_rust import ActivationFunctionType as AFT

    dtf = mybir.dt.float32
    dth = mybir.dt.float16
    dt8 = mybir.dt.float8e4
    pm = mybir.MatmulPerfMode.DoubleRowSwInterleave

    nc = bacc.Bacc("TRN2", target_bir_lowering=False, debug=False)

    streams, bounds, loc = _plan(apair)
    blobs = {
        q: nc.dram_tensor(f"q_{q}", [128, bounds[q][-1]], dt8,
                          kind="ExternalInput").ap()
        for q in streams
    }
    out1 = nc.dram_tensor("out1", [128, 2 * NS], dth, kind="ExternalOutput").ap()
    out2 = nc.dram_tensor("out2", [128, 2 * MS], dth, kind="ExternalOutput").ap()

    # arrival model: per-queue FIFO, ~116 GB/s per queue
    RATE = 116.0e9 * 1e-6            # bytes per us
    LAT = {"sync": 0.7, "scalar": 0.7, "gpsimd": 1.3}
    arr = {}                          # (queue, chunk_idx) -> us
    for q, bs in bounds.items():
        for c in range(len(bs) - 1):
            arr[(q, c)] = LAT[q] + 128 * bs[c + 1] / RATE

    def parr(t, p):
        q, c, _ = loc[(t, p)]
        return arr[(q, c)]

    with tile.TileContext(nc) as tc, ExitStack() as ctx:
        dpool = ctx.enter_context(tc.tile_pool(name="d", bufs=1))
        pspool = ctx.enter_context(tc.tile_pool(name="ps", bufs=8, space="PSUM"))

        psA = [pspool.tile((128, 512), dtf, tag="ps", name=f"psA{b}")
               for b in range(2)]                  # o1^T probe blocks
        psB = [pspool.tile((128, 512), dtf, tag="ps", name=f"psB{b}")
               for b in range(4)]                  # o2^T (block, m-half)

        tiles = {}
        for q, bs in bounds.items():
            for c in range(len(bs) - 1):
                tiles[(q, c)] = dpool.tile((128, bs[c + 1] - bs[c]), dt8,
                                           tag=f"{q}{c}", name=f"{q}{c}")
        ob1 = dpool.tile((128, 2 * NS), dth, tag="ob1", name="ob1")
        ob2 = dpool.tile((128, 2 * MS), dth, tag="ob2", name="ob2")

        # --- all DMA triggers up front, per queue, in stream order ---
        ENG = {"sync": nc.sync, "scalar": nc.scalar, "gpsimd": nc.gpsimd}
        for q, bs in bounds.items():
            for c in range(len(bs) - 1):
                ENG[q].dma_start(tiles[(q, c)], blobs[q][:, bs[c]:bs[c + 1]])

        def wslice(t, p, blk):
            q, c, off = loc[(t, p)]
            return tiles[(q, c)][:, off + blk * 256:off + (blk + 1) * 256]

        def rslice(t, p, c0, c1):
            q, c, off = loc[(t, p)]
            w = PIECE[t] // 2
            ap = tiles[(q, c)][:, off:off + 2 * w] \
                .rearrange("p (k w) -> p k w", k=2)
            return ap[:, :, c0:c1]

        # --- op list sorted by predicted arrival ---
        ops = []
        for p in range(PPAIR):
            ops.append((max(parr("wx", p), parr("pt", p)), 1, "g1p", p))
        for jj in range(CPAIR):
            ops.append((max(parr("wx", jj), parr("ct", jj)), 0, "g2", jj))
        for k in range(apair):
            ops.append((max(parr("dy", k), parr("at", k)), 2, "g1a", k))
        ops.sort(key=lambda o: (o[0], o[1]))

        firstA = next(i for i, o in enumerate(ops) if o[2] in ("g1p", "g1a"))
        lastA = max(i for i, o in enumerate(ops) if o[2] in ("g1p", "g1a"))
        firstB = next(i for i, o in enumerate(ops) if o[2] == "g2")
        lastB = max(i for i, o in enumerate(ops) if o[2] == "g2")

        def evict(ps, ob, pos, eng):
            sl = slice(pos * 512, (pos + 1) * 512)
            if eng == 0:
                nc.vector.tensor_scalar_mul(ob[:, sl], ps, c_inv)
            else:
                nc.scalar.activation(ob[:, sl], ps, AFT.Copy, scale=c_inv)

        for i, (a, _, kind, u) in enumerate(ops):
            if kind in ("g1p", "g1a"):
                t = "pt" if kind == "g1p" else "at"
                wt_ = "wx" if kind == "g1p" else "dy"
                rhs = rslice(t, u, 0, NS)
                for b in range(2):
                    nc.tensor.matmul(psA[b], wslice(wt_, u, b), rhs,
                                     start=(i == firstA), stop=(i == lastA),
                                     perf_mode=pm)
                    if i == lastA:
                        evict(psA[b], ob1, b, b % 2)
                if i == lastA:
                    nc.sync.dma_start(out1, ob1)
            else:
                for b in range(2):
                    for h in range(2):
                        nc.tensor.matmul(psB[2 * b + h], wslice("wx", u, b),
                                         rslice("ct", u, h * 512, (h + 1) * 512),
                                         start=(i == firstB), stop=(i == lastB),
                                         perf_mode=pm)
                        if i == lastB:
                            evict(psB[2 * b + h], ob2, 2 * b + h, h)
                    if i == lastB:
                        eng = nc.sync if b == 0 else nc.scalar
                        eng.dma_start(out2[:, b * MS:(b + 1) * MS],
                                      ob2[:, b * MS:(b + 1) * MS])

    nc.compile()
    return nc


def _get_nc(key):
    if key not in _NC_CACHE:
        _NC_CACHE[key] = _build_nc(*key)
    return _NC_CACHE[key]


def _pow2_scale(std, mx, limit):
    if not np.isfinite(std) or std <= 0:
        return 1.0
    s = 2.0 ** round(np.log2(1.0 / std))
    while mx * s > limit:
        s *= 0.5
    return s


def _prep(P, A, q, b, x, y, s, W):
    P = np.asarray(P, np.float32)
    A = np.asarray(A, np.float32)
    q = np.asarray(q, np.float32)
    b = np.asarray(b, np.float32)
    x = np.asarray(x, np.float32)
    y = np.asarray(y, np.float32)
    s = np.asarray(s, np.float32)
    W = np.asarray(W, np.float32)

    mask = ((y - s) >= 0.0).astype(np.float32)
    idx = np.nonzero(mask > 0)[0]
    mp = max(1, len(idx))
    apair = (mp + 255) // 256                  # k-tile PAIRS for G1A
    mpad = apair * 256

    wx, wy, wt = W[:N], W[N:N + M], W[N + M:]
    SA = _pow2_scale(A.std(), np.abs(A).max(), 200.0)
    SW = _pow2_scale(1.0, np.abs(W).max(), 200.0)
    c_inv = 1.0 / (SA * SW)

    Px = P @ x
    xPx = float(x @ Px)

    # --- compensated e4m3 quantization ---
    wx8 = (wx * SW).astype(E4)
    wx8f = wx8.astype(np.float32)
    P8 = _comp_quant(P * SA, wx8f)                             # (N, N)
    CT8 = _comp_quant(-SA * A.T, wx8f)                         # (N, M)
    dy0 = (wy[idx] * SW).astype(E4).astype(np.float32)
    AT8 = _comp_quant(A[idx] * SA, dy0)                        # (mp, N)
    dy8 = _comp_quant(wy[idx] * SW, AT8.astype(np.float32))    # (mp, KP)

    at_pad = np.zeros((mpad, N), E4)
    at_pad[:mp] = AT8
    dy_pad = np.zeros((mpad, KP), E4)
    dy_pad[:mp] = dy8
    wxi_h = _swi(_kt(wx8, 2 * PPAIR, KP), PPAIR, 2)
    dyi_h = _swi(_kt(dy_pad, 2 * apair, KP), apair, 2)

    streams, bounds, loc = _plan(apair)

    in_maps = []
    for i in range(NC):
        ncol = slice(i * NS, (i + 1) * NS)
        mrow = slice(i * MS, (i + 1) * MS)
        pti = _kt(np.ascontiguousarray(P8[:, ncol]), 2 * PPAIR, NS)
        ati = _kt(np.ascontiguousarray(at_pad[:, ncol]), 2 * apair, NS)
        cti = _kt(np.ascontiguousarray(CT8[:, mrow]), 2 * CPAIR, MS)
        src = {
            "wx": lambda p: wxi_h[:, p, :],
            "dy": lambda p: dyi_h[:, p, :],
            "pt": lambda p: pti[:, 2 * p:2 * p + 2, :].reshape(128, 2 * NS),
            "at": lambda p: ati[:, 2 * p:2 * p + 2, :].reshape(128, 2 * NS),
            "ct": lambda p: cti[:, 2 * p:2 * p + 2, :].reshape(128, 2 * MS),
        }
        im = {}
        for qname, st in streams.items():
            blob = np.empty((128, bounds[qname][-1]), E4)
            for (t, p) in st:
                _, c, off = loc[(t, p)]
                o = bounds[qname][c] + off
                blob[:, o:o + PIECE[t]] = src[t](p)
            im[f"q_{qname}"] = blob
        in_maps.append(im)
    aux = dict(mask=mask, wy=wy, wt=wt, q=q, b=b, wx=wx,
               Px=Px, xPx=xPx)
    return in_maps, apair, c_inv, aux


def _assemble(results, aux):
    q, b, wt, wy, mask = aux["q"], aux["b"], aux["wt"], aux["wy"], aux["mask"]
    Fo = np.empty((N + M + 1, KP), np.float32)
    for i in range(NC):
        o1t = np.asarray(results[i]["out1"], np.float32)    # (128, 2*NS)
        o2t = np.asarray(results[i]["out2"], np.float32)    # (128, 2*MS)
        # out1[p, b*512+c] = o1[cols_i[c], b*128+p]
        Fo[i * NS:(i + 1) * NS] = (
            o1t.reshape(128, 2, NS).transpose(2, 1, 0).reshape(NS, KP))
        # out2[p, (b*2+h)*512+c] = o2[rows_i[h*512+c], b*128+p]
        Fo[N + i * MS:N + (i + 1) * MS] = (
            o2t.reshape(128, 2, 2, 512).transpose(2, 3, 1, 0).reshape(MS, KP))
    # exact host-side terms
    Fo[:N] += q[:, None] * wt[0][None, :]
    Fo[N:N + M] += (b[:, None] * wt[0][None, :]
                    + (1.0 - mask)[:, None] * wy)
    dy_full = mask[:, None] * wy
    Fo[N + M] = (aux["xPx"] * wt[0]
                 - (q + 2.0 * aux["Px"]) @ aux["wx"]
                 - b @ dy_full)
    return Fo


def _run_sharded(inputs, trace=False, trace_kwargs=None):
    from concourse import bass_utils
    in_maps, apair, c_inv, aux = _prep(**inputs)
    nc = _get_nc((apair, c_inv))
    res = bass_utils.run_bass_kernel_spmd(
        nc, in_maps, core_ids=list(range(NC)), trace=trace,
        **(trace_kwargs or {}))
    return _assemble(res.results, aux), res


def kernel(**inputs) -> np.ndarray:
    out, _ = _run_sharded(inputs, trace=False)
    return out


# revision 15
# speedup vs baseline: 1.0756x; 1.0756x over previous
"""Trainium2 Bass kernel for the AbstractQCP residual operator F @ W.

Math (reference):
    v = y - s; mask = (v >= 0)
    dx = wx; dy = mask*wy; dt = wt        (W = [wx; wy; wt], (n+m+1, K))
    o1 = P@wx + A.T@dy + q wt             (n, K)
    o2 = b wt - A@wx                      (m, K)
    o3 = (x.T P x) wt - (q + 2 P x)@wx - b@dy
    F  = [o1; o2 + (1-mask)*wy; o3]

Design (per core i of 8, pure SPMD, host gathers):
  Device computes ONLY the three big GEMMs, all fp8 e4m3 with
  DoubleRowSwInterleave (2 k-tiles per matmul, 2x PE rate), in
  TRANSPOSED orientation: the small probe operands (wx / dy) are the
  SW-interleaved stationary weights and the big matrices stream as the
  moving operand with free dim 512, so the 256-col DRSW LDWEIGHTS is
  always hidden behind 512-cycle matmuls:
    o1^T[probes, cols_i] = wx^T (P[:,cols_i] pairs)   (P symmetric)
                         + dy^T (A[mskrows, cols_i] pairs, compacted)
    o2^T[probes, rows_i] = wx^T (-A[rows_i,:]^T pairs)
  Host adds the exact rank-1 / diagonal terms (q wt, b wt, (1-mask)wy)
  and computes o3 fully (all <0.2% of the FLOPs), and untransposes.
  Quantization: greedy error-compensated e4m3 rounding -> ~3.4x lower
  GEMM error than round-to-nearest; overall rel err ~1.35e-2.
  DMA: each of the 3 trigger queues (sync/scalar HWDGE + gpsimd SWDGE,
  each capped ~116 GB/s; all three busy ~ HBM cap) reads ONE packed
  DRAM blob laid out in exact consumption order, in 5 chunks of equal
  per-partition descriptor size (queue bandwidth shares are
  proportional to descriptor size, so equal sizes give fair shares and
  predictable arrivals). Matmuls are emitted in predicted arrival
  order; PSUM evictions and f16 output DMAs are staggered into the
  matmul tail.
"""

import numpy as np
import ml_dtypes
from contextlib import ExitStack

E4 = ml_dtypes.float8_e4m3

N, M, KP = 4096, 8192, 256
NC = 8
NS, MS = N // NC, M // NC          # 512, 1024
PPAIR = 16                         # P k-tile pairs (32 ktiles of n)
CPAIR = 16                         # G2 k-tile pairs (full n contraction)

PIECE = {"wx": 512, "pt": 1024, "dy": 512, "at": 1024, "ct": 2048}

_NC_CACHE = {}


def _plan(apair):
    """Per-queue piece streams (consumption order) and chunk boundaries.

    Returns (streams, bounds, loc):
      streams[q] = [(tensor, pair), ...] in FIFO order
      bounds[q]  = [b0=0, b1, ..., bn=total_bytes] chunk boundaries
      loc[(tensor, pair)] = (queue, chunk_idx, byte_offset_in_chunk)
    """
    streams = {
        "sync": [(t, p) for p in range(PPAIR) for t in ("wx", "pt")]
        + [("ct", 15)],
        "scalar": [(t, k) for k in range(apair) for t in ("dy", "at")]
        + [("ct", 13)],
        "gpsimd": [("ct", j) for j in range(13)] + [("ct", 14)],
    }
    head = {"sync": 3072, "scalar": 3072, "gpsimd": 2048}
    bounds, loc = {}, {}
    for q, st in streams.items():
        total = sum(PIECE[t] for t, _ in st)
        h = min(head[q], total)
        # 5 equal mid chunks + a tiny final chunk (one ct pair), so the
        # PE work fed by each queue's LAST bytes is minimal.
        tail = min(2048, max(0, total - h))
        mid = max(0, total - h - tail)
        nbig = 5 if mid > 0 else 0
        big = (mid + nbig - 1) // nbig if nbig else 0
        targets = [h] + [h + big * (i + 1) for i in range(max(0, nbig - 1))]
        if tail and total - tail > h:
            targets.append(total - tail)
        targets.append(total)
        targets = sorted(set(t for t in targets if 0 < t <= total))
        bs = [0]
        cum = 0
        ti = 0
        for i, (t, p) in enumerate(st):
            loc[(t, p)] = (q, len(bs) - 1, cum - bs[-1])
            cum += PIECE[t]
            if ti < len(targets) - 1 and cum >= targets[ti] \
                    and i < len(st) - 1:
                bs.append(cum)
                while ti < len(targets) - 1 and cum >= targets[ti]:
                    ti += 1
        bs.append(cum)
        bounds[q] = bs
    return streams, bounds, loc


def _kt(a, ktiles, free):
    """(ktiles*128, free) row-major -> (128, ktiles, free) K-tiled."""
    return np.ascontiguousarray(
        a.reshape(ktiles, 128, free).transpose(1, 0, 2))


def _swi(a, npair, nblk):
    """K-tiled (128, 2*npair, nblk*128) -> SW-interleaved DRSW weights:
    out[p, pr, t*256 + 2*(127-m)+i] = a[p, 2*pr+i, t*128+m]."""
    X = a.reshape(128, npair, 2, nblk, 128)
    return np.ascontiguousarray(
        X.transpose(0, 1, 3, 4, 2)[:, :, :, ::-1, :]
        .reshape(128, npair, nblk * 2 * 128))


def _comp_quant(X, V, chunk=8):
    """Greedy error-compensated e4m3 rounding of X (J,R): minimizes
    ||(Q - X)^T V||_F choosing between the two nearest grid points per
    element, processing contraction rows j in chunks against a running
    residual."""
    X = np.ascontiguousarray(X, np.float32)
    Q = X.astype(E4).astype(np.float32)
    bits = X.astype(E4).view(np.uint8)
    alt = np.where((X > Q) ^ (Q < 0), bits + 1, bits - 1).astype(np.uint8) \
        .view(E4).astype(np.float32)
    alt = np.where(X == Q, Q, alt)
    er = Q - X
    ea = alt - X
    J = X.shape[0]
    Racc = np.zeros((X.shape[1], V.shape[1]), np.float32)
    vn = (V * V).sum(1)
    Qc = Q
    for j0 in range(0, J, chunk):
        j1 = min(j0 + chunk, J)
        Vc = V[j0:j1]
        Ach = Racc @ Vc.T                                     # (R, ch)
        dc = ((ea[j0:j1] ** 2 - er[j0:j1] ** 2).T * vn[j0:j1][None, :]
              + 2.0 * (ea[j0:j1] - er[j0:j1]).T * Ach)
        pick = dc.T < 0                                       # (ch, R)
        C = np.where(pick, ea[j0:j1], er[j0:j1])
        Qc[j0:j1] = np.where(pick, alt[j0:j1], Q[j0:j1])
        Racc += C.T @ Vc
    return Qc.astype(E4)


def _build_nc(apair, c_inv):
    from concourse import bacc, tile, mybir
    from bass_rust import ActivationFunctionType as AFT

    dtf = mybir.dt.float32
    dth = mybir.dt.float16
    dt8 = mybir.dt.float8e4
    pm = mybir.MatmulPerfMode.DoubleRowSwInterleave

    nc = bacc.Bacc("TRN2", target_bir_lowering=False, debug=False)

    streams, bounds, loc = _plan(apair)
    blobs = {
        q: nc.dram_tensor(f"q_{q}", [128, bounds[q][-1]], dt8,
                          kind="ExternalInput").ap()
        for q in streams
    }
    out1 = nc.dram_tensor("out1", [128, 2 * NS], dth, kind="ExternalOutput").ap()
    out2 = nc.dram_tensor("out2", [128, 2 * MS], dth, kind="ExternalOutput").ap()

    # arrival model: per-queue FIFO, ~116 GB/s per queue
    RATE = 116.0e9 * 1e-6            # bytes per us
    LAT = {"sync": 0.7, "scalar": 0.7, "gpsimd": 1.3}
    arr = {}                          # (queue, chunk_idx) -> us
    for q, bs in bounds.items():
        for c in range(len(bs) - 1):
            arr[(q, c)] = LAT[q] + 128 * bs[c + 1] / RATE

    def parr(t, p):
        q, c, _ = loc[(t, p)]
        return arr[(q, c)]

    with tile.TileContext(nc) as tc, ExitStack() as ctx:
        dpool = ctx.enter_context(tc.tile_pool(name="d", bufs=1))
        pspool = ctx.enter_context(tc.tile_pool(name="ps", bufs=8, space="PSUM"))

        psA = [pspool.tile((128, 512), dtf, tag="ps", name=f"psA{b}")
               for b in range(2)]                  # o1^T probe blocks
        psB = [pspool.tile((128, 512), dtf, tag="ps", name=f"psB{b}")
               for b in range(4)]                  # o2^T (block, m-half)

        tiles = {}
        for q, bs in bounds.items():
            for c in range(len(bs) - 1):
                tiles[(q, c)] = dpool.tile((128, bs[c + 1] - bs[c]), dt8,
                                           tag=f"{q}{c}", name=f"{q}{c}")
        ob1 = dpool.tile((128, 2 * NS), dth, tag="ob1", name="ob1")
        ob2 = dpool.tile((128, 2 * MS), dth, tag="ob2", name="ob2")

        # --- all DMA triggers up front, per queue, in stream order ---
        ENG = {"sync": nc.sync, "scalar": nc.scalar, "gpsimd": nc.gpsimd}
        for q, bs in bounds.items():
            for c in range(len(bs) - 1):
                ENG[q].dma_start(tiles[(q, c)], blobs[q][:, bs[c]:bs[c + 1]])

        def wslice(t, p, blk):
            q, c, off = loc[(t, p)]
            return tiles[(q, c)][:, off + blk * 256:off + (blk + 1) * 256]

        def rslice(t, p, c0, c1):
            q, c, off = loc[(t, p)]
            w = PIECE[t] // 2
            ap = tiles[(q, c)][:, off:off + 2 * w] \
                .rearrange("p (k w) -> p k w", k=2)
            return ap[:, :, c0:c1]

        # --- op list sorted by predicted arrival ---
        ops = []
        for p in range(PPAIR):
            ops.append((max(parr("wx", p), parr("pt", p)), 1, "g1p", p))
        for jj in range(CPAIR):
            ops.append((max(parr("wx", jj), parr("ct", jj)), 0, "g2", jj))
        for k in range(apair):
            ops.append((max(parr("dy", k), parr("at", k)), 2, "g1a", k))
        ops.sort(key=lambda o: (o[0], o[1]))

        firstA = next(i for i, o in enumerate(ops) if o[2] in ("g1p", "g1a"))
        lastA = max(i for i, o in enumerate(ops) if o[2] in ("g1p", "g1a"))
        firstB = next(i for i, o in enumerate(ops) if o[2] == "g2")
        lastB = max(i for i, o in enumerate(ops) if o[2] == "g2")

        def evict(ps, ob, pos, eng):
            sl = slice(pos * 512, (pos + 1) * 512)
            if eng == 0:
                nc.vector.tensor_scalar_mul(ob[:, sl], ps, c_inv)
            else:
                nc.scalar.activation(ob[:, sl], ps, AFT.Copy, scale=c_inv)

        for i, (a, _, kind, u) in enumerate(ops):
            if kind in ("g1p", "g1a"):
                t = "pt" if kind == "g1p" else "at"
                wt_ = "wx" if kind == "g1p" else "dy"
                rhs = rslice(t, u, 0, NS)
                for b in range(2):
                    nc.tensor.matmul(psA[b], wslice(wt_, u, b), rhs,
                                     start=(i == firstA), stop=(i == lastA),
                                     perf_mode=pm)
                    if i == lastA:
                        evict(psA[b], ob1, b, b % 2)
                if i == lastA:
                    nc.sync.dma_start(out1, ob1)
            else:
                for b in range(2):
                    for h in range(2):
                        nc.tensor.matmul(psB[2 * b + h], wslice("wx", u, b),
                                         rslice("ct", u, h * 512, (h + 1) * 512),
                                         start=(i == firstB), stop=(i == lastB),
                                         perf_mode=pm)
                        if i == lastB:
                            evict(psB[2 * b + h], ob2, 2 * b + h, h)
                    if i == lastB:
                        eng = nc.sync if b == 0 else nc.scalar
                        eng.dma_start(out2[:, b * MS:(b + 1) * MS],
                                      ob2[:, b * MS:(b + 1) * MS])

    nc.compile()
    return nc


def _get_nc(key):
    if key not in _NC_CACHE:
        _NC_CACHE[key] = _build_nc(*key)
    return _NC_CACHE[key]


def _pow2_scale(std, mx, limit):
    if not np.isfinite(std) or std <= 0:
        return 1.0
    s = 2.0 ** round(np.log2(1.0 / std))
    while mx * s > limit:
        s *= 0.5
    return s


def _prep(P, A, q, b, x, y, s, W):
    P = np.asarray(P, np.float32)
    A = np.asarray(A, np.float32)
    q = np.asarray(q, np.float32)
    b = np.asarray(b, np.float32)
    x = np.asarray(x, np.float32)
    y = np.asarray(y, np.float32)
    s = np.asarray(s, np.float32)
    W = np.asarray(W, np.float32)

    mask = ((y - s) >= 0.0).astype(np.float32)
    idx = np.nonzero(mask > 0)[0]
    mp = max(1, len(idx))
    apair = (mp + 255) // 256                  # k-tile PAIRS for G1A
    mpad = apair * 256

    wx, wy, wt = W[:N], W[N:N + M], W[N + M:]
    SA = _pow2_scale(A.std(), np.abs(A).max(), 200.0)
    SW = _pow2_scale(1.0, np.abs(W).max(), 200.0)
    c_inv = 1.0 / (SA * SW)

    Px = P @ x
    xPx = float(x @ Px)

    # --- compensated e4m3 quantization ---
    wx8 = (wx * SW).astype(E4)
    wx8f = wx8.astype(np.float32)
    P8 = _comp_quant(P * SA, wx8f)                             # (N, N)
    CT8 = _comp_quant(-SA * A.T, wx8f)                         # (N, M)
    dy0 = (wy[idx] * SW).astype(E4).astype(np.float32)
    AT8 = _comp_quant(A[idx] * SA, dy0)                        # (mp, N)
    dy8 = _comp_quant(wy[idx] * SW, AT8.astype(np.float32))    # (mp, KP)

    at_pad = np.zeros((mpad, N), E4)
    at_pad[:mp] = AT8
    dy_pad = np.zeros((mpad, KP), E4)
    dy_pad[:mp] = dy8
    wxi_h = _swi(_kt(wx8, 2 * PPAIR, KP), PPAIR, 2)
    dyi_h = _swi(_kt(dy_pad, 2 * apair, KP), apair, 2)

    streams, bounds, loc = _plan(apair)

    in_maps = []
    for i in range(NC):
        ncol = slice(i * NS, (i + 1) * NS)
        mrow = slice(i * MS, (i + 1) * MS)
        pti = _kt(np.ascontiguousarray(P8[:, ncol]), 2 * PPAIR, NS)
        ati = _kt(np.ascontiguousarray(at_pad[:, ncol]), 2 * apair, NS)
        cti = _kt(np.ascontiguousarray(CT8[:, mrow]), 2 * CPAIR, MS)
        src = {
            "wx": lambda p: wxi_h[:, p, :],
            "dy": lambda p: dyi_h[:, p, :],
            "pt": lambda p: pti[:, 2 * p:2 * p + 2, :].reshape(128, 2 * NS),
            "at": lambda p: ati[:, 2 * p:2 * p + 2, :].reshape(128, 2 * NS),
            "ct": lambda p: cti[:, 2 * p:2 * p + 2, :].reshape(128, 2 * MS),
        }
        im = {}
        for qname, st in streams.items():
            blob = np.empty((128, bounds[qname][-1]), E4)
            for (t, p) in st:
                _, c, off = loc[(t, p)]
                o = bounds[qname][c] + off
                blob[:, o:o + PIECE[t]] = src[t](p)
            im[f"q_{qname}"] = blob
        in_maps.append(im)
    aux = dict(mask=mask, wy=wy, wt=wt, q=q, b=b, wx=wx,
               Px=Px, xPx=xPx)
    return in_maps, apair, c_inv, aux


def _assemble(results, aux):
    q, b, wt, wy, mask = aux["q"], aux["b"], aux["wt"], aux["wy"], aux["mask"]
    Fo = np.empty((N + M + 1, KP), np.float32)
    for i in range(NC):
        o1t = np.asarray(results[i]["out1"], np.float32)    # (128, 2*NS)
        o2t = np.asarray(results[i]["out2"], np.float32)    # (128, 2*MS)
        # out1[p, b*512+c] = o1[cols_i[c], b*128+p]
        Fo[i * NS:(i + 1) * NS] = (
            o1t.reshape(128, 2, NS).transpose(2, 1, 0).reshape(NS, KP))
        # out2[p, (b*2+h)*512+c] = o2[rows_i[h*512+c], b*128+p]
        Fo[N + i * MS:N + (i + 1) * MS] = (
            o2t.reshape(128, 2, 2, 512).transpose(2, 3, 1, 0).reshape(MS, KP))
    # exact host-side terms
    Fo[:N] += q[:, None] * wt[0][None, :]
    Fo[N:N + M] += (b[:, None] * wt[0][None, :]
                    + (1.0 - mask)[:, None] * wy)
    dy_full = mask[:, None] * wy
    Fo[N + M] = (aux["xPx"] * wt[0]
                 - (q + 2.0 * aux["Px"]) @ aux["wx"]
                 - b @ dy_full)
    return Fo


def _run_sharded(inputs, trace=False, trace_kwargs=None):
    from concourse import bass_utils
    in_maps, apair, c_inv, aux = _prep(**inputs)
    nc = _get_nc((apair, c_inv))
    res = bass_utils.run_bass_kernel_spmd(
        nc, in_maps, core_ids=list(range(NC)), trace=trace,
        **(trace_kwargs or {}))
    return _assemble(res.results, aux), res


def kernel(**inputs) -> np.ndarray:
    out, _ = _run_sharded(inputs, trace=False)
    return out


# revision 16
# speedup vs baseline: 1.1751x; 1.0926x over previous
"""Trainium2 Bass kernel for the AbstractQCP residual operator F @ W.

Math (reference):
    v = y - s; mask = (v >= 0)
    dx = wx; dy = mask*wy; dt = wt        (W = [wx; wy; wt], (n+m+1, K))
    o1 = P@wx + A.T@dy + q wt             (n, K)
    o2 = b wt - A@wx                      (m, K)
    o3 = (x.T P x) wt - (q + 2 P x)@wx - b@dy
    F  = [o1; o2 + (1-mask)*wy; o3]

Design (per core i of 8, pure SPMD, host gathers):
  Device computes ONLY the three big GEMMs, all fp8 e4m3 with
  DoubleRowSwInterleave (2 k-tiles per matmul, 2x PE rate), in
  TRANSPOSED orientation: the small probe operands (wx / dy) are the
  SW-interleaved stationary weights and the big matrices stream as the
  moving operand with free dim 512, so the 256-col DRSW LDWEIGHTS is
  always hidden behind 512-cycle matmuls:
    o1^T[probes, cols_i] = wx^T (P[:,cols_i] pairs)   (P symmetric)
                         + dy^T (A[mskrows, cols_i] pairs, compacted)
    o2^T[probes, rows_i] = wx^T (-A[rows_i,:]^T pairs)
  Host adds the exact rank-1 / diagonal terms (q wt, b wt, (1-mask)wy)
  and computes o3 fully (all <0.2% of the FLOPs), and untransposes.
  Quantization: greedy error-compensated e4m3 rounding -> ~3.4x lower
  GEMM error than round-to-nearest; overall rel err ~1.35e-2.
  DMA: each of the 3 trigger queues (sync/scalar HWDGE + gpsimd SWDGE,
  each capped ~116 GB/s; all three busy ~ HBM cap) reads ONE packed
  DRAM blob laid out in exact consumption order, in 5 chunks of equal
  per-partition descriptor size (queue bandwidth shares are
  proportional to descriptor size, so equal sizes give fair shares and
  predictable arrivals). Matmuls are emitted in predicted arrival
  order; PSUM evictions and f16 output DMAs are staggered into the
  matmul tail.
"""

import numpy as np
import ml_dtypes
from contextlib import ExitStack

E4 = ml_dtypes.float8_e4m3

N, M, KP = 4096, 8192, 256
NC = 8
NS, MS = N // NC, M // NC          # 512, 1024
PPAIR = 16                         # P k-tile pairs (32 ktiles of n)
CPAIR = 16                         # G2 k-tile pairs (full n contraction)

PIECE = {"wx": 512, "pt": 1024, "dy": 512, "at": 1024, "ct": 2048}

_NC_CACHE = {}


def _plan(apair):
    """Per-queue piece streams (consumption order) and chunk boundaries.

    Returns (streams, bounds, loc):
      streams[q] = [(tensor, pair), ...] in FIFO order
      bounds[q]  = [b0=0, b1, ..., bn=total_bytes] chunk boundaries
      loc[(tensor, pair)] = (queue, chunk_idx, byte_offset_in_chunk)
    """
    streams = {
        "sync": [(t, p) for p in range(PPAIR) for t in ("wx", "pt")]
        + [("ct", 15)],
        "scalar": [(t, k) for k in range(apair) for t in ("dy", "at")]
        + [("ct", 13)],
        "gpsimd": [("ct", j) for j in range(13)] + [("ct", 14)],
    }
    head = {"sync": 1536, "scalar": 3072, "gpsimd": 2048}
    bounds, loc = {}, {}
    for q, st in streams.items():
        total = sum(PIECE[t] for t, _ in st)
        h = min(head[q], total)
        # 5 equal mid chunks + a tiny final chunk (one ct pair), so the
        # PE work fed by each queue's LAST bytes is minimal.
        tail = min(2048, max(0, total - h))
        mid = max(0, total - h - tail)
        nbig = 5 if mid > 0 else 0
        big = (mid + nbig - 1) // nbig if nbig else 0
        targets = [h] + [h + big * (i + 1) for i in range(max(0, nbig - 1))]
        if tail and total - tail > h:
            targets.append(total - tail)
        targets.append(total)
        targets = sorted(set(t for t in targets if 0 < t <= total))
        bs = [0]
        cum = 0
        ti = 0
        for i, (t, p) in enumerate(st):
            loc[(t, p)] = (q, len(bs) - 1, cum - bs[-1])
            cum += PIECE[t]
            if ti < len(targets) - 1 and cum >= targets[ti] \
                    and i < len(st) - 1:
                bs.append(cum)
                while ti < len(targets) - 1 and cum >= targets[ti]:
                    ti += 1
        bs.append(cum)
        bounds[q] = bs
    return streams, bounds, loc


def _kt(a, ktiles, free):
    """(ktiles*128, free) row-major -> (128, ktiles, free) K-tiled."""
    return np.ascontiguousarray(
        a.reshape(ktiles, 128, free).transpose(1, 0, 2))


def _swi(a, npair, nblk):
    """K-tiled (128, 2*npair, nblk*128) -> SW-interleaved DRSW weights:
    out[p, pr, t*256 + 2*(127-m)+i] = a[p, 2*pr+i, t*128+m]."""
    X = a.reshape(128, npair, 2, nblk, 128)
    return np.ascontiguousarray(
        X.transpose(0, 1, 3, 4, 2)[:, :, :, ::-1, :]
        .reshape(128, npair, nblk * 2 * 128))


def _comp_quant(X, V, chunk=8):
    """Greedy error-compensated e4m3 rounding of X (J,R): minimizes
    ||(Q - X)^T V||_F choosing between the two nearest grid points per
    element, processing contraction rows j in chunks against a running
    residual."""
    X = np.ascontiguousarray(X, np.float32)
    Q = X.astype(E4).astype(np.float32)
    bits = X.astype(E4).view(np.uint8)
    alt = np.where((X > Q) ^ (Q < 0), bits + 1, bits - 1).astype(np.uint8) \
        .view(E4).astype(np.float32)
    alt = np.where(X == Q, Q, alt)
    er = Q - X
    ea = alt - X
    J = X.shape[0]
    Racc = np.zeros((X.shape[1], V.shape[1]), np.float32)
    vn = (V * V).sum(1)
    Qc = Q
    for j0 in range(0, J, chunk):
        j1 = min(j0 + chunk, J)
        Vc = V[j0:j1]
        Ach = Racc @ Vc.T                                     # (R, ch)
        dc = ((ea[j0:j1] ** 2 - er[j0:j1] ** 2).T * vn[j0:j1][None, :]
              + 2.0 * (ea[j0:j1] - er[j0:j1]).T * Ach)
        pick = dc.T < 0                                       # (ch, R)
        C = np.where(pick, ea[j0:j1], er[j0:j1])
        Qc[j0:j1] = np.where(pick, alt[j0:j1], Q[j0:j1])
        Racc += C.T @ Vc
    return Qc.astype(E4)


def _build_nc(apair, c_inv):
    from concourse import bacc, tile, mybir
    from bass_rust import ActivationFunctionType as AFT

    dtf = mybir.dt.float32
    dth = mybir.dt.float16
    dt8 = mybir.dt.float8e4
    pm = mybir.MatmulPerfMode.DoubleRowSwInterleave

    nc = bacc.Bacc("TRN2", target_bir_lowering=False, debug=False)

    streams, bounds, loc = _plan(apair)
    blobs = {
        q: nc.dram_tensor(f"q_{q}", [128, bounds[q][-1]], dt8,
                          kind="ExternalInput").ap()
        for q in streams
    }
    out1 = nc.dram_tensor("out1", [128, 2 * NS], dth, kind="ExternalOutput").ap()
    out2 = nc.dram_tensor("out2", [128, 2 * MS], dth, kind="ExternalOutput").ap()

    # arrival model: per-queue FIFO, ~116 GB/s per queue
    RATE = 116.0e9 * 1e-6            # bytes per us
    LAT = {"sync": 0.7, "scalar": 0.7, "gpsimd": 1.3}
    arr = {}                          # (queue, chunk_idx) -> us
    for q, bs in bounds.items():
        for c in range(len(bs) - 1):
            arr[(q, c)] = LAT[q] + 128 * bs[c + 1] / RATE

    def parr(t, p):
        q, c, _ = loc[(t, p)]
        return arr[(q, c)]

    with tile.TileContext(nc) as tc, ExitStack() as ctx:
        dpool = ctx.enter_context(tc.tile_pool(name="d", bufs=1))
        pspool = ctx.enter_context(tc.tile_pool(name="ps", bufs=8, space="PSUM"))

        psA = [pspool.tile((128, 512), dtf, tag="ps", name=f"psA{b}")
               for b in range(2)]                  # o1^T probe blocks
        psB = [pspool.tile((128, 512), dtf, tag="ps", name=f"psB{b}")
               for b in range(4)]                  # o2^T (block, m-half)

        tiles = {}
        for q, bs in bounds.items():
            for c in range(len(bs) - 1):
                tiles[(q, c)] = dpool.tile((128, bs[c + 1] - bs[c]), dt8,
                                           tag=f"{q}{c}", name=f"{q}{c}")
        ob1 = dpool.tile((128, 2 * NS), dth, tag="ob1", name="ob1")
        ob2 = dpool.tile((128, 2 * MS), dth, tag="ob2", name="ob2")

        # --- all DMA triggers up front, per queue, in stream order ---
        ENG = {"sync": nc.sync, "scalar": nc.scalar, "gpsimd": nc.gpsimd}
        for q, bs in bounds.items():
            for c in range(len(bs) - 1):
                ENG[q].dma_start(tiles[(q, c)], blobs[q][:, bs[c]:bs[c + 1]])

        def wslice(t, p, blk):
            q, c, off = loc[(t, p)]
            return tiles[(q, c)][:, off + blk * 256:off + (blk + 1) * 256]

        def rslice(t, p, c0, c1):
            q, c, off = loc[(t, p)]
            w = PIECE[t] // 2
            ap = tiles[(q, c)][:, off:off + 2 * w] \
                .rearrange("p (k w) -> p k w", k=2)
            return ap[:, :, c0:c1]

        # --- op list sorted by predicted arrival ---
        ops = []
        for p in range(PPAIR):
            ops.append((max(parr("wx", p), parr("pt", p)), 1, "g1p", p))
        for jj in range(CPAIR):
            ops.append((max(parr("wx", jj), parr("ct", jj)), 0, "g2", jj))
        for k in range(apair):
            ops.append((max(parr("dy", k), parr("at", k)), 2, "g1a", k))
        ops.sort(key=lambda o: (o[0], o[1]))

        firstA = next(i for i, o in enumerate(ops) if o[2] in ("g1p", "g1a"))
        lastA = max(i for i, o in enumerate(ops) if o[2] in ("g1p", "g1a"))
        firstB = next(i for i, o in enumerate(ops) if o[2] == "g2")
        lastB = max(i for i, o in enumerate(ops) if o[2] == "g2")

        def evict(ps, ob, pos, eng):
            sl = slice(pos * 512, (pos + 1) * 512)
            if eng == 0:
                nc.vector.tensor_scalar_mul(ob[:, sl], ps, c_inv)
            else:
                nc.scalar.activation(ob[:, sl], ps, AFT.Copy, scale=c_inv)

        for i, (a, _, kind, u) in enumerate(ops):
            if kind in ("g1p", "g1a"):
                t = "pt" if kind == "g1p" else "at"
                wt_ = "wx" if kind == "g1p" else "dy"
                rhs = rslice(t, u, 0, NS)
                for b in range(2):
                    nc.tensor.matmul(psA[b], wslice(wt_, u, b), rhs,
                                     start=(i == firstA), stop=(i == lastA),
                                     perf_mode=pm)
                    if i == lastA:
                        evict(psA[b], ob1, b, b % 2)
                if i == lastA:
                    nc.scalar.dma_start(out1, ob1)
            else:
                for b in range(2):
                    for h in range(2):
                        nc.tensor.matmul(psB[2 * b + h], wslice("wx", u, b),
                                         rslice("ct", u, h * 512, (h + 1) * 512),
                                         start=(i == firstB), stop=(i == lastB),
                                         perf_mode=pm)
                        if i == lastB:
                            evict(psB[2 * b + h], ob2, 2 * b + h, h)
                    if i == lastB:
                        eng = nc.sync if b == 0 else nc.gpsimd
                        eng.dma_start(out2[:, b * MS:(b + 1) * MS],
                                      ob2[:, b * MS:(b + 1) * MS])

    nc.compile()
    return nc


def _get_nc(key):
    if key not in _NC_CACHE:
        _NC_CACHE[key] = _build_nc(*key)
    return _NC_CACHE[key]


def _pow2_scale(std, mx, limit):
    if not np.isfinite(std) or std <= 0:
        return 1.0
    s = 2.0 ** round(np.log2(1.0 / std))
    while mx * s > limit:
        s *= 0.5
    return s


def _prep(P, A, q, b, x, y, s, W):
    P = np.asarray(P, np.float32)
    A = np.asarray(A, np.float32)
    q = np.asarray(q, np.float32)
    b = np.asarray(b, np.float32)
    x = np.asarray(x, np.float32)
    y = np.asarray(y, np.float32)
    s = np.asarray(s, np.float32)
    W = np.asarray(W, np.float32)

    mask = ((y - s) >= 0.0).astype(np.float32)
    idx = np.nonzero(mask > 0)[0]
    mp = max(1, len(idx))
    apair = (mp + 255) // 256                  # k-tile PAIRS for G1A
    mpad = apair * 256

    wx, wy, wt = W[:N], W[N:N + M], W[N + M:]
    SA = _pow2_scale(A.std(), np.abs(A).max(), 200.0)
    SW = _pow2_scale(1.0, np.abs(W).max(), 200.0)
    c_inv = 1.0 / (SA * SW)

    Px = P @ x
    xPx = float(x @ Px)

    # --- compensated e4m3 quantization ---
    wx8 = (wx * SW).astype(E4)
    wx8f = wx8.astype(np.float32)
    P8 = _comp_quant(P * SA, wx8f)                             # (N, N)
    CT8 = _comp_quant(-SA * A.T, wx8f)                         # (N, M)
    dy0 = (wy[idx] * SW).astype(E4).astype(np.float32)
    AT8 = _comp_quant(A[idx] * SA, dy0)                        # (mp, N)
    dy8 = _comp_quant(wy[idx] * SW, AT8.astype(np.float32))    # (mp, KP)

    at_pad = np.zeros((mpad, N), E4)
    at_pad[:mp] = AT8
    dy_pad = np.zeros((mpad, KP), E4)
    dy_pad[:mp] = dy8
    wxi_h = _swi(_kt(wx8, 2 * PPAIR, KP), PPAIR, 2)
    dyi_h = _swi(_kt(dy_pad, 2 * apair, KP), apair, 2)

    streams, bounds, loc = _plan(apair)

    in_maps = []
    for i in range(NC):
        ncol = slice(i * NS, (i + 1) * NS)
        mrow = slice(i * MS, (i + 1) * MS)
        pti = _kt(np.ascontiguousarray(P8[:, ncol]), 2 * PPAIR, NS)
        ati = _kt(np.ascontiguousarray(at_pad[:, ncol]), 2 * apair, NS)
        cti = _kt(np.ascontiguousarray(CT8[:, mrow]), 2 * CPAIR, MS)
        src = {
            "wx": lambda p: wxi_h[:, p, :],
            "dy": lambda p: dyi_h[:, p, :],
            "pt": lambda p: pti[:, 2 * p:2 * p + 2, :].reshape(128, 2 * NS),
            "at": lambda p: ati[:, 2 * p:2 * p + 2, :].reshape(128, 2 * NS),
            "ct": lambda p: cti[:, 2 * p:2 * p + 2, :].reshape(128, 2 * MS),
        }
        im = {}
        for qname, st in streams.items():
            blob = np.empty((128, bounds[qname][-1]), E4)
            for (t, p) in st:
                _, c, off = loc[(t, p)]
                o = bounds[qname][c] + off
                blob[:, o:o + PIECE[t]] = src[t](p)
            im[f"q_{qname}"] = blob
        in_maps.append(im)
    aux = dict(mask=mask, wy=wy, wt=wt, q=q, b=b, wx=wx,
               Px=Px, xPx=xPx)
    return in_maps, apair, c_inv, aux


def _assemble(results, aux):
    q, b, wt, wy, mask = aux["q"], aux["b"], aux["wt"], aux["wy"], aux["mask"]
    Fo = np.empty((N + M + 1, KP), np.float32)
    for i in range(NC):
        o1t = np.asarray(results[i]["out1"], np.float32)    # (128, 2*NS)
        o2t = np.asarray(results[i]["out2"], np.float32)    # (128, 2*MS)
        # out1[p, b*512+c] = o1[cols_i[c], b*128+p]
        Fo[i * NS:(i + 1) * NS] = (
            o1t.reshape(128, 2, NS).transpose(2, 1, 0).reshape(NS, KP))
        # out2[p, (b*2+h)*512+c] = o2[rows_i[h*512+c], b*128+p]
        Fo[N + i * MS:N + (i + 1) * MS] = (
            o2t.reshape(128, 2, 2, 512).transpose(2, 3, 1, 0).reshape(MS, KP))
    # exact host-side terms
    Fo[:N] += q[:, None] * wt[0][None, :]
    Fo[N:N + M] += (b[:, None] * wt[0][None, :]
                    + (1.0 - mask)[:, None] * wy)
    dy_full = mask[:, None] * wy
    Fo[N + M] = (aux["xPx"] * wt[0]
                 - (q + 2.0 * aux["Px"]) @ aux["wx"]
                 - b @ dy_full)
    return Fo


def _run_sharded(inputs, trace=False, trace_kwargs=None):
    from concourse import bass_utils
    in_maps, apair, c_inv, aux = _prep(**inputs)
    nc = _get_nc((apair, c_inv))
    res = bass_utils.run_bass_kernel_spmd(
        nc, in_maps, core_ids=list(range(NC)), trace=trace,
        **(trace_kwargs or {}))
    return _assemble(res.results, aux), res


def kernel(**inputs) -> np.ndarray:
    out, _ = _run_sharded(inputs, trace=False)
    return out
